# revision 1
# baseline (speedup 1.0000x reference)
"""Trainium2 Bass kernel for nn_CrossAttentionBlock (B=4, C=512, H=W=64).

Decomposition across 8 NeuronCores: core = (batch b, query-half h).
Each core:
  stage 1: theta/phi = conv1x1(x1) packed as one [128-out] projection (PE)
  stage 2: g^T = conv1x1(x0) in [m, 64] layout + ones column (PE)
  main:    fT[m, n] = theta^T phi (PE, keys on partitions), p = exp(fT) (ACT),
           yT_ext = [g, 1]^T p accumulated over key chunks (PE) -> softmax
           numerator rows 0..63 and denominator row 64 in one accumulation.
  gather:  transpose yT -> y rows, normalize by denominator, + g_b,
           pair-wise AllGather assembles the full y for the batch.
  phase 2: W_y = W [view of y] consumed only as per-channel bn stats (AdaIN
           needs only mean/var of W_y); x0 instance stats; final out =
           r * x0 + t with per-channel scalars.

SPMD uniformity: the key/spatial axis m and the channel axis c are dummy
(contraction/stat) indices, so each core receives inputs permuted so that
"its" queries and "its" output channels come first; the host un-permutes
the output columns.
"""
import numpy as np
from contextlib import ExitStack

import concourse.bass as bass
import concourse.tile as tile
from concourse import mybir
from concourse.bass_utils import run_bass_kernel_spmd

FP32 = mybir.dt.float32
ALU = mybir.AluOpType
ACTF = mybir.ActivationFunctionType

B, C, H, W = 4, 512, 64, 64
N = H * W          # 4096 tokens
C8 = C // 8        # 64 inner channels
NH = N // 2        # 2048 queries per core
OC = C // 2        # 256 output channels per core
EPS = 1e-5

REPLICA_PAIRS = [[0, 1], [2, 3], [4, 5], [6, 7]]


def _split_excess_waits(nc, max_waits=1, drain_max=1):
    """walrus here rejects instructions carrying more than ~2 sync waits; move
    extras to preceding NoOps on the same engine (semantics preserved: waits
    run before the instruction, engine streams are sequential)."""
    for blk in nc.main_func.blocks:
        insts = blk.instructions
        k = 0
        while k < len(insts):
            inst = insts[k]
            si = inst.sync_info
            cap = drain_max if inst.opcode == "Drain" else max_waits
            if si is not None and si.on_wait and len(si.on_wait) > cap:
                waits = list(si.on_wait)
                keep = waits[-cap:]
                extra = waits[:-cap]
                pos = k
                for j in range(0, len(extra), cap):
                    nop = mybir.InstNoOp(name=f"{inst.name}-wsplit{j}", ins=[], outs=[])
                    nop.engine = inst.engine
                    nop.sync_info = mybir.SyncInfo(
                        on_wait=extra[j : j + cap], on_update=[]
                    )
                    insts.insert(pos, nop)
                    pos += 1
                    k += 1
                inst.sync_info = mybir.SyncInfo(on_wait=keep, on_update=list(si.on_update))
            k += 1


def build_nc():
    nc = bass.Bass()

    x0 = nc.dram_tensor("x0", [C, N], FP32, kind="ExternalInput")
    x1 = nc.dram_tensor("x1", [C, N], FP32, kind="ExternalInput")
    tp_wT = nc.dram_tensor("tp_wT", [C, 128], FP32, kind="ExternalInput")
    tp_b = nc.dram_tensor("tp_b", [128, 1], FP32, kind="ExternalInput")
    g_wT = nc.dram_tensor("g_wT", [C, C8], FP32, kind="ExternalInput")
    g_b_bc = nc.dram_tensor("g_b_bc", [128, C8], FP32, kind="ExternalInput")
    W_wTh = nc.dram_tensor("W_wTh", [C8, OC], FP32, kind="ExternalInput")
    W_bh = nc.dram_tensor("W_bh", [128, 2], FP32, kind="ExternalInput")
    ident = nc.dram_tensor("ident", [C8 + 1, C8 + 1], FP32, kind="ExternalInput")
    out = nc.dram_tensor("out", [OC, N], FP32, kind="ExternalOutput")

    y_bounce = nc.dram_tensor("y_bounce", [NH, C8], FP32)
    y_full = nc.dram_tensor("y_full", [N, C8], FP32)

    with tile.TileContext(nc) as tc, ExitStack() as ctx:
        wpool = ctx.enter_context(tc.tile_pool(name="weights", bufs=1))
        big = ctx.enter_context(tc.tile_pool(name="big", bufs=1))

        # ---- weights to SBUF ----
        tp_w_sb = wpool.tile([128, 4, 128], FP32)
        g_w_sb = wpool.tile([128, 4, C8], FP32)
        for c in range(4):
            nc.sync.dma_start(out=tp_w_sb[:, c, :], in_=tp_wT[c * 128:(c + 1) * 128, :])
            nc.sync.dma_start(out=g_w_sb[:, c, :], in_=g_wT[c * 128:(c + 1) * 128, :])
        tp_b_sb = wpool.tile([128, 1], FP32)
        nc.sync.dma_start(out=tp_b_sb[:], in_=tp_b[:])
        g_b_sb = wpool.tile([128, C8], FP32)
        nc.sync.dma_start(out=g_b_sb[:], in_=g_b_bc[:])
        W_w_sb = wpool.tile([C8, OC], FP32)
        nc.sync.dma_start(out=W_w_sb[:], in_=W_wTh[:])
        W_b_sb = wpool.tile([128, 2], FP32)
        nc.sync.dma_start(out=W_b_sb[:], in_=W_bh[:])
        id_sb = wpool.tile([C8 + 1, C8 + 1], FP32)
        nc.sync.dma_start(out=id_sb[:], in_=ident[:])

        # ---- persistent big tensors ----
        x0_sb = big.tile([128, 4, N], FP32)      # c-chunk on middle index
        theta_sb = big.tile([C8, N], FP32)       # keys, [64, 4096]
        phi_sb = big.tile([C8, NH], FP32)        # queries (own half), [64, 2048]
        g_extT = big.tile([128, 32, C8 + 1], FP32)  # [m-chunk, 65] per chunk
        yT_sb = big.tile([C8 + 1, NH], FP32)
        yv_sb = big.tile([C8, N], FP32)          # gathered y viewed [64, 4096]

        nc.gpsimd.memset(g_extT[:, :, C8:C8 + 1], 1.0)

        # ---- stage 1: x1 -> theta/phi ----
        with tc.tile_pool(name="x1blk", bufs=8) as x1pool, \
             tc.tile_pool(name="ps_tp", bufs=2, space="PSUM") as ps_tp:
            for blk in range(8):
                cols = slice(blk * 512, (blk + 1) * 512)
                xt = []
                for c in range(4):
                    t = x1pool.tile([128, 512], FP32)
                    nc.sync.dma_start(out=t[:], in_=x1[c * 128:(c + 1) * 128, cols])
                    xt.append(t)
                ptp = ps_tp.tile([128, 512], FP32)
                for c in range(4):
                    nc.tensor.matmul(ptp[:], tp_w_sb[:, c, :], xt[c][:],
                                     start=(c == 0), stop=(c == 3))
                nc.vector.tensor_scalar_add(theta_sb[:, cols], ptp[0:C8, :],
                                            tp_b_sb[0:C8, :])
                if blk < 4:
                    nc.vector.tensor_scalar_add(phi_sb[:, cols], ptp[C8:128, :],
                                                tp_b_sb[C8:128, :])

        # ---- stage 2: x0 -> g^T (transposed layout) ----
        with tc.tile_pool(name="ps_g", bufs=2, space="PSUM") as ps_g:
            for blk in range(8):
                cols = slice(blk * 512, (blk + 1) * 512)
                for c in range(4):
                    nc.sync.dma_start(out=x0_sb[:, c, cols],
                                      in_=x0[c * 128:(c + 1) * 128, cols])
                for mi in range(4 * blk, 4 * blk + 4):
                    pg = ps_g.tile([128, C8], FP32)
                    for c in range(4):
                        nc.tensor.matmul(pg[:],
                                         x0_sb[:, c, mi * 128:(mi + 1) * 128],
                                         g_w_sb[:, c, :],
                                         start=(c == 0), stop=(c == 3))
                    nc.vector.tensor_copy(g_extT[:, mi, 0:C8], pg[:])

        # ---- x0 instance stats (own channels = chunks 0, 1) ----
        stat = ctx.enter_context(tc.tile_pool(name="stats", bufs=1))
        x_aggs = []
        for oc in range(2):
            xst = stat.tile([128, 8, 6], FP32)
            for mb in range(8):
                nc.vector.bn_stats(xst[:, mb, :],
                                   x0_sb[:, oc, mb * 512:(mb + 1) * 512])
            xagg = stat.tile([128, 2], FP32)
            nc.vector.bn_aggr(xagg[:], xst[:])
            x_aggs.append(xagg)

        # ---- main attention loop ----
        with tc.tile_pool(name="ps_f", bufs=2, space="PSUM") as ps_f, \
             tc.tile_pool(name="ps_y", bufs=1, space="PSUM") as ps_y, \
             tc.tile_pool(name="pT", bufs=3) as ppool:
            for q in range(2):
                qc = slice(q * 1024, (q + 1) * 1024)
                py = ps_y.tile([C8 + 1, 1024], FP32)
                for mi in range(32):
                    ft = ps_f.tile([128, 1024], FP32)
                    for s in range(2):
                        nc.tensor.matmul(
                            ft[:, s * 512:(s + 1) * 512],
                            theta_sb[:, mi * 128:(mi + 1) * 128],
                            phi_sb[:, q * 1024 + s * 512: q * 1024 + (s + 1) * 512],
                            start=True, stop=True)
                    pt = ppool.tile([128, 1024], FP32)
                    nc.scalar.activation(pt[:], ft[:], ACTF.Exp)
                    for s in range(2):
                        nc.tensor.matmul(
                            py[:, s * 512:(s + 1) * 512],
                            g_extT[:, mi, :],
                            pt[:, s * 512:(s + 1) * 512],
                            start=(mi == 0), stop=(mi == 31))
                nc.vector.tensor_copy(yT_sb[:, qc], py[:])

        # ---- transpose, normalize, exchange ----
        with tc.tile_pool(name="ps_t", bufs=2, space="PSUM") as ps_t, \
             tc.tile_pool(name="ystage", bufs=3) as ystage:
            for j in range(16):
                ptile = ps_t.tile([128, C8 + 1], FP32)
                nc.tensor.transpose(ptile[:], yT_sb[:, j * 128:(j + 1) * 128], id_sb[:])
                rec = ystage.tile([128, 1], FP32, tag="rec")
                nc.vector.reciprocal(rec[:], ptile[:, C8:C8 + 1])
                yst = ystage.tile([128, C8], FP32, tag="yst")
                nc.vector.tensor_scalar_mul(yst[:], ptile[:, 0:C8], rec[:])
                nc.vector.tensor_add(yst[:], yst[:], g_b_sb[:])
                nc.sync.dma_start(out=y_bounce[j * 128:(j + 1) * 128, :], in_=yst[:])

        nc.gpsimd.collective_compute(
            "AllGather", ALU.bypass,
            replica_groups=REPLICA_PAIRS,
            ins=[y_bounce[:]],
            outs=[y_full[:]],
        )
        nc.sync.dma_start(out=yv_sb[:],
                          in_=y_full[:].rearrange("(a b) w -> a (b w)", a=C8))

        # ---- phase 2: W_y stats + per-channel affine + output ----
        with tc.tile_pool(name="ps_W", bufs=2, space="PSUM") as ps_W, \
             tc.tile_pool(name="sc", bufs=1) as sc, \
             tc.tile_pool(name="outp", bufs=2) as outp:
            for oc in range(2):
                wst = sc.tile([128, 8, 6], FP32, tag=f"wst{oc}")
                for mb in range(8):
                    pw = ps_W.tile([128, 512], FP32)
                    nc.tensor.matmul(pw[:], W_w_sb[:, oc * 128:(oc + 1) * 128],
                                     yv_sb[:, mb * 512:(mb + 1) * 512],
                                     start=True, stop=True)
                    nc.vector.bn_stats(wst[:, mb, :], pw[:])
                wagg = sc.tile([128, 2], FP32, tag=f"wagg{oc}")
                nc.vector.bn_aggr(wagg[:], wst[:])

                # r = sqrt((var_s + eps) / (var_c + eps)); t = mu_s - r*mu_c
                vc = sc.tile([128, 1], FP32, tag=f"vc{oc}")
                nc.vector.tensor_scalar_add(vc[:], x_aggs[oc][:, 1:2], EPS)
                rc = sc.tile([128, 1], FP32, tag=f"rc{oc}")
                nc.vector.reciprocal(rc[:], vc[:])
                vs = sc.tile([128, 1], FP32, tag=f"vs{oc}")
                nc.vector.tensor_scalar_add(vs[:], wagg[:, 1:2], EPS)
                ratio = sc.tile([128, 1], FP32, tag=f"ratio{oc}")
                nc.vector.tensor_mul(ratio[:], vs[:], rc[:])
                rr = sc.tile([128, 1], FP32, tag=f"rr{oc}")
                nc.scalar.sqrt(rr[:], ratio[:])
                mus = sc.tile([128, 1], FP32, tag=f"mus{oc}")
                nc.vector.tensor_add(mus[:], wagg[:, 0:1], W_b_sb[:, oc:oc + 1])
                rmc = sc.tile([128, 1], FP32, tag=f"rmc{oc}")
                nc.vector.tensor_mul(rmc[:], rr[:], x_aggs[oc][:, 0:1])
                tt = sc.tile([128, 1], FP32, tag=f"tt{oc}")
                nc.vector.tensor_sub(tt[:], mus[:], rmc[:])

                for mb in range(4):
                    cols = slice(mb * 1024, (mb + 1) * 1024)
                    ot = outp.tile([128, 1024], FP32)
                    nc.vector.tensor_scalar(ot[:], x0_sb[:, oc, cols], rr[:], tt[:],
                                            ALU.mult, ALU.add)
                    nc.sync.dma_start(out=out[oc * 128:(oc + 1) * 128, cols], in_=ot[:])

    _split_excess_waits(nc)
    return nc


_NC_CACHE = None


def _get_nc():
    global _NC_CACHE
    if _NC_CACHE is None:
        _NC_CACHE = build_nc()
    return _NC_CACHE


def _core_inputs(x0f, x1f, tp_wT, tp_b, g_wT, g_b, W_wT, W_b, ident, core):
    b, half = core // 2, core % 2
    x0b, x1b = x0f[b], x1f[b]
    if half == 0:
        x0p = x0b
        x1p = x1b
        g_wp = g_wT
    else:
        # queries-first column permutation; own-channels-first row permutation
        x1p = np.concatenate([x1b[:, NH:], x1b[:, :NH]], axis=1)
        x0r = np.concatenate([x0b[OC:], x0b[:OC]], axis=0)
        x0p = np.concatenate([x0r[:, NH:], x0r[:, :NH]], axis=1)
        g_wp = np.concatenate([g_wT[OC:], g_wT[:OC]], axis=0)
    return {
        "x0": np.ascontiguousarray(x0p),
        "x1": np.ascontiguousarray(x1p),
        "tp_wT": tp_wT,
        "tp_b": tp_b,
        "g_wT": np.ascontiguousarray(g_wp),
        "g_b_bc": np.ascontiguousarray(np.broadcast_to(g_b, (128, C8))),
        "W_wTh": np.ascontiguousarray(W_wT[:, half * OC:(half + 1) * OC]),
        "W_bh": np.ascontiguousarray(
            W_b[half * OC:(half + 1) * OC].reshape(2, 128).T),
        "ident": ident,
    }


def kernel(x0, x1, g_w, g_b, theta_w, theta_b, phi_w, phi_b, W_w, W_b):
    x0 = np.asarray(x0, dtype=np.float32)
    x1 = np.asarray(x1, dtype=np.float32)
    x0f = x0.reshape(B, C, N)
    x1f = x1.reshape(B, C, N)
    tp_wT = np.ascontiguousarray(
        np.concatenate([theta_w, phi_w], axis=0).T.astype(np.float32))
    tp_b = np.ascontiguousarray(
        np.concatenate([theta_b, phi_b]).astype(np.float32)[:, None])
    g_wT = np.ascontiguousarray(np.asarray(g_w, np.float32).T)
    W_wT = np.ascontiguousarray(np.asarray(W_w, np.float32).T)
    ident = np.eye(C8 + 1, dtype=np.float32)
    g_b = np.asarray(g_b, np.float32)
    W_b = np.asarray(W_b, np.float32)

    in_maps = [
        _core_inputs(x0f, x1f, tp_wT, tp_b, g_wT, g_b, W_wT, W_b, ident, core)
        for core in range(8)
    ]
    nc = _get_nc()
    res = run_bass_kernel_spmd(nc, in_maps, core_ids=list(range(8)))

    out = np.empty((B, C, N), dtype=np.float32)
    for core in range(8):
        b, half = core // 2, core % 2
        o = res.results[core]["out"]
        if half == 1:
            o = np.concatenate([o[:, NH:], o[:, :NH]], axis=1)
        out[b, half * OC:(half + 1) * OC] = o
    return out.reshape(B, C, H, W)



# revision 8
# speedup vs baseline: 1.8915x; 1.8915x over previous
"""Trainium2 Bass kernel for nn_CrossAttentionBlock (B=4, C=512, H=W=64).

Decomposition across 8 NeuronCores: core = (batch b, query-half h).
All heavy matmuls run in bf16 (1 cycle/row on the PE vs 4 for fp32);
the harness tolerance (2e-2) leaves orders of magnitude of margin since
AdaIN only consumes aggregate statistics of the attention output.

Per core:
  prologue (streamed per 512-column block as DMA lands):
    theta/phi = conv1x1(x1) packed as one 128-row projection -> bf16
    g         = conv1x1(x0) + g_b (folded: softmax-normalizing (g+b)
                equals normalizing g then adding b) -> bf16,
                PE-transposed into g_extT [keys, 65] with a ones column
  main loop (mi = key-chunk outer, query-half inner):
    fT = theta^T phi (PE), p = exp(fT) (ACT, bf16 out),
    yT_ext += [g,1]^T p (PE)  -> numerator rows 0..63, denominator row 64
  tail: transpose yT, scale by 1/denominator, pair AllGather (bf16),
    W_y consumed only as per-channel bn stats, final out = r*x0 + t
    (per-channel scalars) on the Pool engine.

SPMD uniformity: inputs are host-permuted so each core's queries and
output channels come first; the host un-permutes output columns.
"""
import numpy as np
from contextlib import ExitStack

import ml_dtypes

import concourse.bass as bass
import concourse.tile as tile
from concourse import mybir
from concourse.bass_utils import run_bass_kernel_spmd

FP32 = mybir.dt.float32
BF16 = mybir.dt.bfloat16
ALU = mybir.AluOpType
ACTF = mybir.ActivationFunctionType

B, C, H, W = 4, 512, 64, 64
N = H * W          # 4096 tokens
C8 = C // 8        # 64 inner channels
NH = N // 2        # 2048 queries per core
OC = C // 2        # 256 output channels per core
EPS = 1e-5

REPLICA_PAIRS = [[0, 1], [2, 3], [4, 5], [6, 7]]

NPBF16 = ml_dtypes.bfloat16


def _split_excess_waits(nc, max_waits=1, drain_max=1):
    """walrus here rejects instructions carrying more than ~2 sync waits; move
    extras to preceding NoOps on the same engine (semantics preserved: waits
    run before the instruction, engine streams are sequential)."""
    for blk in nc.main_func.blocks:
        insts = blk.instructions
        k = 0
        while k < len(insts):
            inst = insts[k]
            si = inst.sync_info
            cap = drain_max if inst.opcode == "Drain" else max_waits
            if si is not None and si.on_wait and len(si.on_wait) > cap:
                waits = list(si.on_wait)
                keep = waits[-cap:]
                extra = waits[:-cap]
                pos = k
                for j in range(0, len(extra), cap):
                    nop = mybir.InstNoOp(name=f"{inst.name}-wsplit{j}", ins=[], outs=[])
                    nop.engine = inst.engine
                    nop.sync_info = mybir.SyncInfo(
                        on_wait=extra[j : j + cap], on_update=[]
                    )
                    insts.insert(pos, nop)
                    pos += 1
                    k += 1
                inst.sync_info = mybir.SyncInfo(on_wait=keep, on_update=list(si.on_update))
            k += 1


def build_nc():
    nc = bass.Bass()

    x0 = nc.dram_tensor("x0", [C, N], BF16, kind="ExternalInput")
    x1 = nc.dram_tensor("x1", [C, N], BF16, kind="ExternalInput")
    tp_wT = nc.dram_tensor("tp_wT", [C, 128], BF16, kind="ExternalInput")
    tp_b = nc.dram_tensor("tp_b", [128, 1], FP32, kind="ExternalInput")
    g_wT = nc.dram_tensor("g_wT", [C, C8], BF16, kind="ExternalInput")
    g_b64 = nc.dram_tensor("g_b64", [C8, 1], FP32, kind="ExternalInput")
    W_wTh = nc.dram_tensor("W_wTh", [C8, OC], BF16, kind="ExternalInput")
    W_bh = nc.dram_tensor("W_bh", [128, 2], FP32, kind="ExternalInput")
    ident = nc.dram_tensor("ident", [C8 + 1, C8 + 1], FP32, kind="ExternalInput")
    out = nc.dram_tensor("out", [OC, N], FP32, kind="ExternalOutput")

    y_bounce = nc.dram_tensor("y_bounce", [NH, C8], BF16)
    y_full = nc.dram_tensor("y_full", [N, C8], BF16)

    with tile.TileContext(nc) as tc, ExitStack() as ctx:
        wpool = ctx.enter_context(tc.tile_pool(name="weights", bufs=1))
        big = ctx.enter_context(tc.tile_pool(name="big", bufs=1))

        # ---- weights to SBUF ----
        tp_w_sb = wpool.tile([128, 4, 128], BF16)
        nc.sync.dma_start(out=tp_w_sb[:], in_=tp_wT[:].rearrange("(c p) o -> p c o", c=4))
        g_w_sb = wpool.tile([128, 4, C8], BF16)
        nc.sync.dma_start(out=g_w_sb[:], in_=g_wT[:].rearrange("(c p) o -> p c o", c=4))
        tp_b_sb = wpool.tile([128, 1], FP32)
        nc.sync.dma_start(out=tp_b_sb[:], in_=tp_b[:])
        g_b_sb = wpool.tile([C8, 1], FP32)
        nc.sync.dma_start(out=g_b_sb[:], in_=g_b64[:])
        W_w_sb = wpool.tile([C8, OC], BF16)
        nc.sync.dma_start(out=W_w_sb[:], in_=W_wTh[:])
        W_b_sb = wpool.tile([128, 2], FP32)
        nc.sync.dma_start(out=W_b_sb[:], in_=W_bh[:])
        id_sb = wpool.tile([C8 + 1, C8 + 1], FP32)
        nc.sync.dma_start(out=id_sb[:], in_=ident[:])

        # ---- persistent big tensors ----
        x0_sb = big.tile([128, 4, N], BF16)      # c-chunk on middle index
        x1_sb = big.tile([128, 4, N], BF16)
        theta_sb = big.tile([C8, N], BF16)       # keys, [64, 4096]
        phi_sb = big.tile([C8, NH], BF16)        # queries (own half), [64, 2048]
        g_extT = big.tile([128, 32, C8 + 1], BF16)  # [m-chunk, 65] per chunk
        yT_sb = big.tile([C8 + 1, NH], FP32)
        ys_sb = big.tile([128, 16, C8], BF16)    # normalized y, token-major
        yv_sb = big.tile([C8, N], BF16)          # gathered y viewed [64, 4096]

        nc.gpsimd.memset(g_extT[:, :, C8:C8 + 1], 1.0)

        # ---- input DMA: x1 blocks 0-3 / x0 blocks 0-3 interleaved, then
        # x0 4-7 (g prologue), then x1 4-7 (theta for far keys) ----
        def load_block(t_dram, t_sb, b):
            cols = slice(b * 512, (b + 1) * 512)
            nc.sync.dma_start(
                out=t_sb[:, :, cols],
                in_=t_dram[:, cols].rearrange("(c p) n -> p c n", c=4))

        for b in range(4):
            load_block(x1, x1_sb, b)
            load_block(x0, x0_sb, b)
        for b in range(4, 8):
            load_block(x0, x0_sb, b)
        for b in range(4, 8):
            load_block(x1, x1_sb, b)

        # ---- prologue per block: theta/phi proj, g proj, g transposes ----
        ps_big = ctx.enter_context(tc.tile_pool(name="ps_big", bufs=2, space="PSUM"))
        gstage = ctx.enter_context(tc.tile_pool(name="gstage", bufs=2))

        def proj_block(b):
            cols = slice(b * 512, (b + 1) * 512)
            # theta/phi packed projection
            pp = ps_big.tile([128, 1024], FP32, tag="ps")
            for c in range(4):
                nc.tensor.matmul(pp[:, 0:512], tp_w_sb[:, c, :],
                                 x1_sb[:, c, cols], start=(c == 0), stop=(c == 3))
            nc.vector.tensor_scalar_add(theta_sb[:, cols], pp[0:C8, 0:512],
                                        tp_b_sb[0:C8, :])
            if b < 4:
                nc.vector.tensor_scalar_add(phi_sb[:, cols], pp[C8:128, 0:512],
                                            tp_b_sb[C8:128, :])
            # g projection (+ folded g_b)
            pg = ps_big.tile([128, 1024], FP32, tag="ps")
            for c in range(4):
                nc.tensor.matmul(pg[0:C8, 0:512], g_w_sb[:, c, :],
                                 x0_sb[:, c, cols], start=(c == 0), stop=(c == 3))
            g_blk = gstage.tile([C8, 512], FP32, tag="g")
            nc.vector.tensor_scalar_add(g_blk[:], pg[0:C8, 0:512], g_b_sb[:])
            # transpose 4 key-chunks of 128 into g_extT
            tr = ps_big.tile([128, 1024], FP32, tag="ps")
            for k in range(4):
                nc.tensor.transpose(tr[:, k * 64:(k + 1) * 64],
                                    g_blk[:, k * 128:(k + 1) * 128],
                                    id_sb[0:C8, 0:C8])
            nc.vector.tensor_copy(g_extT[:, 4 * b:4 * b + 4, 0:C8], tr[:, 0:256])

        for b in range(4):
            proj_block(b)

        # ---- x0 instance stats (own channels = chunks 0, 1) ----
        stat = ctx.enter_context(tc.tile_pool(name="stats", bufs=1))
        xst = [stat.tile([128, 8, 6], FP32, tag=f"xst{oc}", name=f"xst{oc}") for oc in range(2)]

        def x0_stats_block(b):
            for oc in range(2):
                nc.vector.bn_stats(xst[oc][:, b, :],
                                   x0_sb[:, oc, b * 512:(b + 1) * 512])

        for b in range(4):
            x0_stats_block(b)

        # ---- main attention loop (mi outer, query-half inner) ----
        pend = {14: 4, 18: 5, 22: 6, 26: 7}   # late prologue insertion points
        ps_y = ctx.enter_context(tc.tile_pool(name="ps_y", bufs=1, space="PSUM"))
        ptpool = ctx.enter_context(tc.tile_pool(name="pT", bufs=3))
        py = [ps_y.tile([C8 + 1, 1024], FP32, tag=f"py{q}", name=f"py{q}") for q in range(2)]

        for mi in range(32):
            if mi in pend:
                b = pend[mi]
                proj_block(b)
                x0_stats_block(b)
            for q in range(2):
                ft = ps_big.tile([128, 1024], FP32, tag="ps")
                for s in range(2):
                    nc.tensor.matmul(
                        ft[:, s * 512:(s + 1) * 512],
                        theta_sb[:, mi * 128:(mi + 1) * 128],
                        phi_sb[:, q * 1024 + s * 512: q * 1024 + (s + 1) * 512],
                        start=True, stop=True)
                pt = ptpool.tile([128, 1024], BF16, tag="pt")
                nc.scalar.activation(pt[:], ft[:], ACTF.Exp)
                for s in range(2):
                    nc.tensor.matmul(
                        py[q][:, s * 512:(s + 1) * 512],
                        g_extT[:, mi, :],
                        pt[:, s * 512:(s + 1) * 512],
                        start=(mi == 0), stop=(mi == 31))

        xagg = [stat.tile([128, 2], FP32, tag=f"xagg{oc}", name=f"xagg{oc}") for oc in range(2)]
        for oc in range(2):
            nc.vector.bn_aggr(xagg[oc][:], xst[oc][:])

        # ---- transpose + normalize + exchange ----
        with tc.tile_pool(name="ystage", bufs=2) as ystage:
            for q in range(2):
                nc.vector.tensor_copy(yT_sb[:, q * 1024:(q + 1) * 1024], py[q][:])
            for j in range(16):
                ptile = ps_big.tile([128, 1024], FP32, tag="ps")
                nc.tensor.transpose(ptile[:, 0:C8 + 1],
                                    yT_sb[:, j * 128:(j + 1) * 128], id_sb[:])
                rec = ystage.tile([128, 1], FP32, tag="rec")
                nc.vector.reciprocal(rec[:], ptile[:, C8:C8 + 1])
                nc.vector.tensor_scalar_mul(ys_sb[:, j, :], ptile[:, 0:C8], rec[:])
            nc.sync.dma_start(
                out=y_bounce[:].rearrange("(j p) w -> p j w", p=128),
                in_=ys_sb[:])

        nc.gpsimd.collective_compute(
            "AllGather", ALU.bypass,
            replica_groups=REPLICA_PAIRS,
            ins=[y_bounce[:]],
            outs=[y_full[:]],
        )
        nc.sync.dma_start(out=yv_sb[:],
                          in_=y_full[:].rearrange("(a b) w -> a (b w)", a=C8))

        # ---- phase 2: W_y stats + per-channel affine + output ----
        with tc.tile_pool(name="sc", bufs=1) as sc, \
             tc.tile_pool(name="outp", bufs=2) as outp:
            for oc in range(2):
                wst = sc.tile([128, 4, 2, 6], FP32, tag=f"wst{oc}")
                for mb in range(4):
                    pw = ps_big.tile([128, 1024], FP32, tag="ps")
                    for s in range(2):
                        nc.tensor.matmul(
                            pw[:, s * 512:(s + 1) * 512],
                            W_w_sb[:, oc * 128:(oc + 1) * 128],
                            yv_sb[:, mb * 1024 + s * 512: mb * 1024 + (s + 1) * 512],
                            start=True, stop=True)
                    for s in range(2):
                        nc.vector.bn_stats(wst[:, mb, s, :],
                                           pw[:, s * 512:(s + 1) * 512])
                wagg = sc.tile([128, 2], FP32, tag=f"wagg{oc}")
                nc.vector.bn_aggr(wagg[:], wst[:])

                # r = sqrt((var_s + eps) / (var_c + eps)); t = mu_s - r*mu_c
                vc = sc.tile([128, 1], FP32, tag=f"vc{oc}")
                nc.vector.tensor_scalar_add(vc[:], xagg[oc][:, 1:2], EPS)
                rc = sc.tile([128, 1], FP32, tag=f"rc{oc}")
                nc.vector.reciprocal(rc[:], vc[:])
                vs = sc.tile([128, 1], FP32, tag=f"vs{oc}")
                nc.vector.tensor_scalar_add(vs[:], wagg[:, 1:2], EPS)
                ratio = sc.tile([128, 1], FP32, tag=f"ratio{oc}")
                nc.vector.tensor_mul(ratio[:], vs[:], rc[:])
                rr = sc.tile([128, 1], FP32, tag=f"rr{oc}")
                nc.scalar.sqrt(rr[:], ratio[:])
                mus = sc.tile([128, 1], FP32, tag=f"mus{oc}")
                nc.vector.tensor_add(mus[:], wagg[:, 0:1], W_b_sb[:, oc:oc + 1])
                rmc = sc.tile([128, 1], FP32, tag=f"rmc{oc}")
                nc.vector.tensor_mul(rmc[:], rr[:], xagg[oc][:, 0:1])
                tt = sc.tile([128, 1], FP32, tag=f"tt{oc}")
                nc.vector.tensor_sub(tt[:], mus[:], rmc[:])

                for mb in range(4):
                    cols = slice(mb * 1024, (mb + 1) * 1024)
                    ot = outp.tile([128, 1024], FP32, tag="ot")
                    nc.gpsimd.tensor_scalar(ot[:], x0_sb[:, oc, cols], rr[:], tt[:],
                                            ALU.mult, ALU.add)
                    nc.sync.dma_start(out=out[oc * 128:(oc + 1) * 128, cols], in_=ot[:])

    _split_excess_waits(nc)
    return nc


_NC_CACHE = None


def _get_nc():
    global _NC_CACHE
    if _NC_CACHE is None:
        _NC_CACHE = build_nc()
    return _NC_CACHE


def _core_inputs(x0f, x1f, tp_wT, tp_b, g_wT, g_b, W_wT, W_b, ident, core):
    b, half = core // 2, core % 2
    x0b, x1b = x0f[b], x1f[b]
    if half == 0:
        x0p = x0b
        x1p = x1b
        g_wp = g_wT
    else:
        # queries-first column permutation; own-channels-first row permutation
        x1p = np.concatenate([x1b[:, NH:], x1b[:, :NH]], axis=1)
        x0r = np.concatenate([x0b[OC:], x0b[:OC]], axis=0)
        x0p = np.concatenate([x0r[:, NH:], x0r[:, :NH]], axis=1)
        g_wp = np.concatenate([g_wT[OC:], g_wT[:OC]], axis=0)
    return {
        "x0": np.ascontiguousarray(x0p),
        "x1": np.ascontiguousarray(x1p),
        "tp_wT": tp_wT,
        "tp_b": tp_b,
        "g_wT": np.ascontiguousarray(g_wp),
        "g_b64": g_b,
        "W_wTh": np.ascontiguousarray(W_wT[:, half * OC:(half + 1) * OC]),
        "W_bh": np.ascontiguousarray(
            W_b[half * OC:(half + 1) * OC].reshape(2, 128).T),
        "ident": ident,
    }


def _prepare_in_maps(x0, x1, g_w, g_b, theta_w, theta_b, phi_w, phi_b, W_w, W_b):
    x0f = np.asarray(x0, np.float32).reshape(B, C, N).astype(NPBF16)
    x1f = np.asarray(x1, np.float32).reshape(B, C, N).astype(NPBF16)
    tp_wT = np.ascontiguousarray(
        np.concatenate([theta_w, phi_w], axis=0).T.astype(NPBF16))
    tp_b = np.ascontiguousarray(
        np.concatenate([theta_b, phi_b]).astype(np.float32)[:, None])
    g_wT = np.ascontiguousarray(np.asarray(g_w, np.float32).T.astype(NPBF16))
    W_wT = np.ascontiguousarray(np.asarray(W_w, np.float32).T.astype(NPBF16))
    ident = np.eye(C8 + 1, dtype=np.float32)
    g_b = np.ascontiguousarray(np.asarray(g_b, np.float32)[:, None])
    W_b = np.asarray(W_b, np.float32)
    return [
        _core_inputs(x0f, x1f, tp_wT, tp_b, g_wT, g_b, W_wT, W_b, ident, core)
        for core in range(8)
    ]


def kernel(x0, x1, g_w, g_b, theta_w, theta_b, phi_w, phi_b, W_w, W_b):
    in_maps = _prepare_in_maps(x0, x1, g_w, g_b, theta_w, theta_b,
                               phi_w, phi_b, W_w, W_b)
    nc = _get_nc()
    res = run_bass_kernel_spmd(nc, in_maps, core_ids=list(range(8)))

    out = np.empty((B, C, N), dtype=np.float32)
    for core in range(8):
        b, half = core // 2, core % 2
        o = res.results[core]["out"]
        if half == 1:
            o = np.concatenate([o[:, NH:], o[:, :NH]], axis=1)
        out[b, half * OC:(half + 1) * OC] = o
    return out.reshape(B, C, H, W)
